# revision 1
# baseline (speedup 1.0000x reference)
"""Block-diagonal 2x2 equalizer kernel for Trainium2 (8 NeuronCores).

Per point (b, u, s, f) solves the 2x2 system M x = v by Cramer's rule:
    m_ij = h[b, pi[u], i, 0, 2u+j, s, f]   (only 1/4 of h is needed)
    det  = m00*m11 - m01*m10
    x0   = (m11*v0 - m01*v1) / det
    x1   = (m00*v1 - m10*v0) / det
    out[b, u, a, s, f] = x_a

Sharding: data-parallel over batch, 2 batches per core on 8 cores. The host
gathers (precoding_ind) and packs operand planes into contiguous [128, fd]
blocks so every device DMA is a large fully-contiguous transfer.

Device kernel is raw Bass (no TileContext): the neuronxcc walrus used by the
axon/bass2jax path allows only one sync-wait per instruction, so all waits
are standalone wait_ge instructions and every SBUF buffer is written exactly
once (pure dataflow, per-chunk semaphores, no WAR hazards, no tail barrier).

Pipeline (NCH chunks over the u axis):
  sync engine:  per chunk, loads A={m00|m11}, B={m01|m10}, Y={v0|v1}
  DVE:          all 11 tensor ops per chunk (p0, p1, det, q0, q1, r0, q2,
                q3, r1, x0, x1). GPSIMD is intentionally UNUSED: measured
                on HW, concurrent GPSIMD+DVE contend for SBUF ports and
                drop combined throughput below DVE alone (DVE TT 1.09us
                -> 2.9us while GPSIMD runs).
  ACT (scalar): rdet = Reciprocal(det) via direct InstActivation (HW
                spline measured 2.2e-5 max rel err, 1.04us vs 5.75us for
                DVE reciprocal at FD=896); also issues the stores
"""

from contextlib import ExitStack

import numpy as np

import concourse.bass as bass
import concourse.mybir as mybir
from concourse.bass_utils import run_bass_kernel_spmd

# Problem shapes (hardcoded per contract)
B, U, A, NTX, T, S, F = 16, 4, 2, 1, 8, 14, 2048
SF = S * F               # 28672
NCORES = 8
BPC = B // NCORES        # 2 batches per core
NCH = 2                  # pipeline chunks (groups of u)
UPC = U // NCH           # u's per chunk
QW = 448                 # inner width: SF = 64 * 448
ROWS = SF // QW          # 64 rows -> partition p = b*64 + row
FD = UPC * QW            # free elems per component per chunk

# Set by test harness to capture an NTFF profile on the run.
TRACE = False
LAST_RESULTS = None


def _pack(d):
    """[BPC, U, SF] -> [NCH, 128, FD] with p = b*ROWS + sf//QW, f = ul*QW + sf%QW."""
    d = d.reshape(BPC, U, ROWS, QW)
    out = np.empty((NCH, BPC * ROWS, FD), np.float32)
    for k in range(NCH):
        blk = d[:, k * UPC:(k + 1) * UPC]               # [BPC, UPC, ROWS, QW]
        out[k] = blk.transpose(0, 2, 1, 3).reshape(BPC * ROWS, FD)
    return out


def _unpack(t):
    """Inverse of _pack: [NCH, 128, FD] -> [BPC, U, SF]."""
    out = np.empty((BPC, U, ROWS, QW), np.float32)
    for k in range(NCH):
        blk = t[k].reshape(BPC, ROWS, UPC, QW).transpose(0, 2, 1, 3)
        out[:, k * UPC:(k + 1) * UPC] = blk
    return out.reshape(BPC, U, SF)


def _build_nc():
    f32 = mybir.dt.float32
    nc = bass.Bass("TRN2")
    # hA: [m00 | m11], hB: [m01 | m10], yB: [v0 | v1], xout: [x0 | x1]
    hA = nc.dram_tensor("hA", [NCH, 128, 2 * FD], f32, kind="ExternalInput")
    hB = nc.dram_tensor("hB", [NCH, 128, 2 * FD], f32, kind="ExternalInput")
    yB = nc.dram_tensor("yB", [NCH, 128, 2 * FD], f32, kind="ExternalInput")
    xout = nc.dram_tensor("xout", [NCH, 128, 2 * FD], f32, kind="ExternalOutput")

    with ExitStack() as ctx:
        tA = [ctx.enter_context(nc.sbuf_tensor(f"tA{k}", [128, 2 * FD], f32)) for k in range(NCH)]
        tB = [ctx.enter_context(nc.sbuf_tensor(f"tB{k}", [128, 2 * FD], f32)) for k in range(NCH)]
        tY = [ctx.enter_context(nc.sbuf_tensor(f"tY{k}", [128, 2 * FD], f32)) for k in range(NCH)]
        tX = [ctx.enter_context(nc.sbuf_tensor(f"tX{k}", [128, 2 * FD], f32)) for k in range(NCH)]
        tp = [
            {
                n: ctx.enter_context(nc.sbuf_tensor(f"{n}_{k}", [128, FD], f32))
                for n in ("p0", "p1", "q2", "q3", "det", "rdet", "q0", "q1", "r0", "r1")
            }
            for k in range(NCH)
        ]
        semA = [ctx.enter_context(nc.semaphore(f"semA{k}")) for k in range(NCH)]
        semB = [ctx.enter_context(nc.semaphore(f"semB{k}")) for k in range(NCH)]
        semY = [ctx.enter_context(nc.semaphore(f"semY{k}")) for k in range(NCH)]
        semO = [ctx.enter_context(nc.semaphore(f"semO{k}")) for k in range(NCH)]
        dve_sem = ctx.enter_context(nc.semaphore("dve_sem"))
        act_sem = ctx.enter_context(nc.semaphore("act_sem"))

        with nc.Block() as block:

            @block.sync
            def _(sync):
                for k in range(NCH):
                    sync.dma_start(out=tA[k][:], in_=hA[k]).then_inc(semA[k], 16)
                    sync.dma_start(out=tY[k][:], in_=yB[k]).then_inc(semY[k], 16)
                    sync.dma_start(out=tB[k][:], in_=hB[k]).then_inc(semB[k], 16)

            # dve_sem counts: chunk k ops are 11k+1 .. 11k+11
            # order chosen so every consumer is >=2 ops after its producers
            # (a wait_ge on a just-finished DVE op stalls ~1-2us for the
            # producer's pipe DRAIN + sem propagation; with distance the
            # waits are already satisfied): p0 q0 q2 p1 q1 q3 det r0 r1 x0 x1
            @block.vector
            def _(vector):
                for k in range(NCH):
                    a, b, y, x, t = tA[k], tB[k], tY[k], tX[k], tp[k]
                    m00, m11 = a[:, :FD], a[:, FD:]
                    m01, m10 = b[:, :FD], b[:, FD:]
                    v0, v1 = y[:, :FD], y[:, FD:]
                    c = 11 * k
                    vector.wait_ge(semA[k], 16)
                    vector.tensor_mul(t["p0"][:], m00, m11).then_inc(dve_sem, 1)   # c+1
                    vector.wait_ge(semY[k], 16)
                    vector.tensor_mul(t["q0"][:], m11, v0).then_inc(dve_sem, 1)    # c+2
                    vector.tensor_mul(t["q2"][:], m00, v1).then_inc(dve_sem, 1)    # c+3
                    vector.wait_ge(semB[k], 16)
                    vector.tensor_mul(t["p1"][:], m01, m10).then_inc(dve_sem, 1)   # c+4
                    vector.tensor_mul(t["q1"][:], m01, v1).then_inc(dve_sem, 1)    # c+5
                    vector.tensor_mul(t["q3"][:], m10, v0).then_inc(dve_sem, 1)    # c+6
                    vector.wait_ge(dve_sem, c + 4)
                    vector.tensor_sub(t["det"][:], t["p0"][:], t["p1"][:]).then_inc(
                        dve_sem, 1
                    )  # c+7  (ACT recip consumes)
                    vector.wait_ge(dve_sem, c + 5)
                    vector.tensor_sub(t["r0"][:], t["q0"][:], t["q1"][:]).then_inc(
                        dve_sem, 1
                    )  # c+8
                    vector.wait_ge(dve_sem, c + 6)
                    vector.tensor_sub(t["r1"][:], t["q2"][:], t["q3"][:]).then_inc(
                        dve_sem, 1
                    )  # c+9
                    vector.wait_ge(dve_sem, c + 8)
                    vector.wait_ge(act_sem, k + 1)
                    vector.tensor_mul(x[:, :FD], t["r0"][:], t["rdet"][:]).then_inc(
                        dve_sem, 1
                    )  # c+10 (store x0 consumes)
                    vector.wait_ge(dve_sem, c + 9)
                    vector.tensor_mul(x[:, FD:], t["r1"][:], t["rdet"][:]).then_inc(
                        dve_sem, 1
                    )  # c+11 (store x1 consumes)

            @block.scalar
            def _(scalar):
                for k in range(NCH):
                    c = 11 * k
                    scalar.wait_ge(dve_sem, c + 7)
                    scalar.add_instruction(
                        mybir.InstActivation(
                            name=nc.get_next_instruction_name(),
                            func=mybir.ActivationFunctionType.Reciprocal,
                            ins=[
                                scalar.lower_ap(tp[k]["det"][:]),
                                mybir.ImmediateValue(dtype=f32, value=0.0),
                                mybir.ImmediateValue(dtype=f32, value=1.0),
                                mybir.ImmediateValue(dtype=f32, value=0.0),
                            ],
                            outs=[scalar.lower_ap(tp[k]["rdet"][:])],
                        )
                    ).then_inc(act_sem, 1)
                    scalar.wait_ge(dve_sem, c + 10)
                    scalar.dma_start(out=xout[k, :, :FD], in_=tX[k][:, :FD]).then_inc(
                        semO[k], 16
                    )
                    scalar.wait_ge(dve_sem, c + 11)
                    scalar.dma_start(out=xout[k, :, FD:], in_=tX[k][:, FD:]).then_inc(
                        semO[k], 16
                    )
                for k in range(NCH):
                    scalar.wait_ge(semO[k], 32)

    return nc


def make_in_maps(y, h, precoding_ind):
    """Host-side gather + pack. Returns per-core input maps."""
    y = np.asarray(y)
    h = np.asarray(h)
    pi = np.asarray(precoding_ind).astype(np.int64)

    hg = h[:, pi[0]]                                     # [B, U, A, NTX, T, S, F]
    # hsel[b, u, i, j] = hg[b, u, i, 0, 2u+j]  -> components c = i*2+j
    hsel = np.stack(
        [hg[:, u, :, 0, 2 * u:2 * u + 2] for u in range(U)], axis=1
    )                                                    # [B, U, A(i), 2(j), S, F]
    hsel = np.ascontiguousarray(hsel).reshape(B, U, 4, SF).astype(np.float32)
    yr = np.ascontiguousarray(y).reshape(B, U, A, SF).astype(np.float32)

    in_maps = []
    for c in range(NCORES):
        b0 = c * BPC
        hs = hsel[b0:b0 + BPC]                           # [BPC, U, 4, SF]
        ys = yr[b0:b0 + BPC]                             # [BPC, U, A, SF]
        hA = np.concatenate([_pack(hs[:, :, 0]), _pack(hs[:, :, 3])], axis=2)
        hB = np.concatenate([_pack(hs[:, :, 1]), _pack(hs[:, :, 2])], axis=2)
        yB = np.concatenate([_pack(ys[:, :, 0]), _pack(ys[:, :, 1])], axis=2)
        in_maps.append({
            "hA": np.ascontiguousarray(hA),
            "hB": np.ascontiguousarray(hB),
            "yB": np.ascontiguousarray(yB),
        })
    return in_maps


def assemble_output(results):
    """Per-core xout [NCH, 128, 2FD] -> full [B, U, A, S, F]."""
    out = np.empty((B, U, A, S, F), np.float32)
    for c in range(NCORES):
        xo = np.asarray(results[c]["xout"])
        x0 = _unpack(xo[:, :, :FD]).reshape(BPC, U, S, F)
        x1 = _unpack(xo[:, :, FD:]).reshape(BPC, U, S, F)
        out[c * BPC:(c + 1) * BPC, :, 0] = x0
        out[c * BPC:(c + 1) * BPC, :, 1] = x1
    return out


def kernel(y, h, precoding_ind):
    global LAST_RESULTS
    in_maps = make_in_maps(y, h, precoding_ind)
    nc = _build_nc()
    res = run_bass_kernel_spmd(nc, in_maps, list(range(NCORES)), trace=TRACE)
    LAST_RESULTS = res
    return assemble_output(res.results)

